# revision 5
# baseline (speedup 1.0000x reference)
"""DICE/NLL 3D loss kernel for Trainium2 (8 NeuronCores, data-parallel over X).

Reference computation (see problem):
    logp  = log_softmax(output, axis=1)            # [B, C, X, Y, Z]
    picked = take_along_axis(logp, mask, axis=1)   # [B, 1, X, Y, Z]
    loss = sum over (B, Z) of -mean over (X, Y) of picked
         = (1 / (X*Y)) * sum_pixels [ logsumexp_C(x) - x_mask ]

Device strategy (per core, X sharded 8 ways):
  - x shipped as bf16 [B*C, PIX]; mask shipped as bf16 [B, PIX]
  - ACT: e = exp(x) (bf16), per-class
  - PE : s = sum_c e_c via identity-weight matmuls accumulating in PSUM (f32)
  - ACT: Ln(s) with accum_out -> per-partition running sums of lse
  - DVE: one-hot masks mu_c = (m == c) in bf16
  - PE : acc[q,n] += sum_p mu_c[p,q] * x_c[p,n] for 128-col blocks; the PSUM
         diagonal accumulates sum_pixels x_mask (host takes the trace)
  - host: total = (sum lse_acc - trace(acc_pick)) / (X*Y), summed over cores
"""

import os

import numpy as np


# Problem constants (hardcoded per contract; kernel.py must be self-contained).
B, C, X, Y, Z = 2, 4, 256, 256, 64
NCORES = 8
XS = X // NCORES          # 32 x-planes per core
PIX = XS * Y * Z          # 524288 pixels per (b, c) per core
MT = 2048                 # macro-tile free dim (per class)
MPIX = 128 * MT           # 262144 pixels per macro tile
NJ = PIX // MPIX          # 2 chunks per batch
NMT = B * NJ              # 4 macro tiles per core
QH = 1024                 # PSUM tile free dim for the softmax-denominator path
BLK = 128                 # block width for the mask-select matmuls
NLSE = NMT * (MT // QH)   # number of lse accumulator columns

_BF16 = np.float16

_cache: dict = {}


def _build_nc(repeat=None):
    """Build and compile the per-core Bass program (same NEFF for all cores).

    repeat: if set, wrap the whole computation in a hardware For_i loop that
    recomputes the same result `repeat` times — used only for timing (the
    outputs are identical; dispatch overhead amortizes away).
    """
    import contextlib

    import concourse.bacc as bacc
    import concourse.mybir as mybir
    import concourse.tile as tile

    f32 = mybir.dt.float32
    bf16 = mybir.dt.float16
    AF = mybir.ActivationFunctionType
    ALU = mybir.AluOpType

    nc = bacc.Bacc("TRN2", target_bir_lowering=False, debug=False)

    x_dram = nc.dram_tensor("x", [B * C, PIX], bf16, kind="ExternalInput")
    m_dram = nc.dram_tensor("m", [B, PIX], bf16, kind="ExternalInput")
    id_dram = nc.dram_tensor("ident", [128, 128], bf16, kind="ExternalInput")
    pick_dram = nc.dram_tensor("pick", [128, 128], f32, kind="ExternalOutput")
    lse_dram = nc.dram_tensor("lse", [128, NLSE], f32, kind="ExternalOutput")

    with tile.TileContext(nc) as tc:
        with (
            tc.tile_pool(name="xp", bufs=2) as xp,
            tc.tile_pool(name="mp", bufs=2) as mp,
            tc.tile_pool(name="ep", bufs=2) as ep,
            tc.tile_pool(name="up", bufs=2) as up,
            tc.tile_pool(name="scr", bufs=2) as scr,
            tc.tile_pool(name="cons", bufs=1) as cons,
            tc.tile_pool(name="outp", bufs=1) as outp,
            tc.tile_pool(name="sps", bufs=2, space="PSUM") as sps,
            tc.tile_pool(name="accps", bufs=1, space="PSUM") as accps,
        ):
            ident = cons.tile([128, 128], bf16)
            nc.sync.dma_start(ident[:], id_dram[:])

            acc_pick = accps.tile([128, 128], f32)
            lse_acc = outp.tile([128, NLSE], f32)

            loop_cm = (
                tc.For_i(0, repeat, 1) if repeat else contextlib.nullcontext()
            )
            with loop_cm:
                _emit_body(
                    nc, mybir, xp, mp, ep, up, scr, sps, outp,
                    x_dram, m_dram, ident, acc_pick, lse_acc,
                    pick_dram, lse_dram,
                )

    nc.compile()
    return nc


def _emit_body(
    nc, mybir, xp, mp, ep, up, scr, sps, outp,
    x_dram, m_dram, ident, acc_pick, lse_acc, pick_dram, lse_dram,
):
    f32 = mybir.dt.float32
    bf16 = mybir.dt.float16
    AF = mybir.ActivationFunctionType
    ALU = mybir.AluOpType

    if True:
        if True:
            nmm = NMT * (MT // BLK) * C
            mmi = 0
            for t in range(NMT):
                b, j = divmod(t, NJ)
                xt = xp.tile([128, C * MT], bf16, name=f"xt{t}", tag="xt")
                for c in range(C):
                    src = x_dram[b * C + c, j * MPIX : (j + 1) * MPIX]
                    nc.sync.dma_start(
                        xt[:, c * MT : (c + 1) * MT],
                        src.rearrange("(p f) -> p f", p=128),
                    )
                mt_ = mp.tile([128, MT], bf16, name=f"mt{t}", tag="mt")
                nc.sync.dma_start(
                    mt_[:],
                    m_dram[b, j * MPIX : (j + 1) * MPIX].rearrange(
                        "(p f) -> p f", p=128
                    ),
                )

                et = ep.tile([128, C * MT], bf16, name=f"et{t}", tag="et")
                nc.scalar.activation(et[:], xt[:], AF.Exp)

                ut = up.tile([128, C * MT], bf16, name=f"ut{t}", tag="ut")
                for c in range(C):
                    nc.vector.tensor_scalar(
                        ut[:, c * MT : (c + 1) * MT],
                        mt_[:],
                        float(c),
                        None,
                        op0=ALU.is_equal,
                    )

                for h in range(MT // QH):
                    s_ps = sps.tile([128, QH], f32, name=f"s{t}_{h}", tag="s")
                    for q in range(QH // 512):
                        for c in range(C):
                            nc.tensor.matmul(
                                s_ps[:, q * 512 : (q + 1) * 512],
                                ident[:],
                                et[
                                    :,
                                    c * MT + h * QH + q * 512 : c * MT
                                    + h * QH
                                    + (q + 1) * 512,
                                ],
                                start=(c == 0),
                                stop=(c == C - 1),
                            )
                    lscr = scr.tile([128, QH], bf16, name=f"l{t}_{h}", tag="l")
                    col = t * (MT // QH) + h
                    nc.scalar.activation(
                        lscr[:],
                        s_ps[:],
                        AF.Ln,
                        accum_out=lse_acc[:, col : col + 1],
                    )

                for blk in range(MT // BLK):
                    for c in range(C):
                        lo = c * MT + blk * BLK
                        nc.tensor.matmul(
                            acc_pick[:],
                            ut[:, lo : lo + BLK],
                            xt[:, lo : lo + BLK],
                            start=(mmi == 0),
                            stop=(mmi == nmm - 1),
                            skip_group_check=True,
                        )
                        mmi += 1

            pick_sb = outp.tile([128, 128], f32)
            nc.vector.tensor_copy(pick_sb[:], acc_pick[:])
            nc.sync.dma_start(pick_dram[:], pick_sb[:])
            nc.sync.dma_start(lse_dram[:], lse_acc[:])


def _get_nc():
    if "nc" not in _cache:
        try:
            import jax

            cache_dir = os.environ.get(
                "KERNEL_JAX_CACHE_DIR", os.path.expanduser("~/.dice3d_jax_cache")
            )
            os.makedirs(cache_dir, exist_ok=True)
            jax.config.update("jax_compilation_cache_dir", cache_dir)
            jax.config.update("jax_persistent_cache_min_entry_size_bytes", -1)
            jax.config.update("jax_persistent_cache_min_compile_time_secs", 0.1)
        except Exception:
            pass
        _cache["nc"] = _build_nc()
    return _cache["nc"]


def make_in_maps(output: np.ndarray, mask: np.ndarray):
    """Shard + cast the full inputs into the 8 per-core input maps."""
    xr = np.ascontiguousarray(output).reshape(B, C, NCORES, PIX)
    mr = np.ascontiguousarray(mask).reshape(B, NCORES, PIX)
    ident = np.eye(128, dtype=_BF16)
    in_maps = []
    for k in range(NCORES):
        xk = xr[:, :, k, :].astype(_BF16).reshape(B * C, PIX)
        mk = mr[:, k, :].astype(_BF16)
        in_maps.append({"x": xk, "m": mk, "ident": ident})
    return in_maps


def combine_results(results) -> np.ndarray:
    """results: list of per-core {"pick": [128,128] f32, "lse": [128,NLSE] f32}."""
    total = 0.0
    for r in results:
        total += float(r["lse"].astype(np.float64).sum())
        total -= float(np.trace(r["pick"].astype(np.float64)))
    return np.asarray(total / (X * Y), dtype=np.float32)


def kernel(output: np.ndarray, mask: np.ndarray) -> np.ndarray:
    from concourse import bass_utils

    nc = _get_nc()
    in_maps = make_in_maps(output, mask)
    res = bass_utils.run_bass_kernel_spmd(nc, in_maps, core_ids=list(range(NCORES)))
    return combine_results(res.results)
